# revision 13
# baseline (speedup 1.0000x reference)
"""Trainium2 Bass kernel for nn_CriticHead (critic head over C*t tasks).

Contract: kernel(**inputs) takes the FULL unsharded inputs (as produced by
setup_inputs()) and returns the FULL [1, T] float32 output.  Internally the
work is sharded data-parallel over the leading cluster axis across 8
NeuronCores; the tiny MLP weights are replicated.

Math (per task j, verified against the reference):
    me_j   = mean(enode[j,:])                       # since y41 = y2 * me
    sc_j   = sum(ccl[j,:]) * sum(cnd[j,:])          # since y42 = y2 * sc
    u_j    = [outer3(res_j, fr_j, estep_j) (150) ; bb_j (768)]   # 918
    y2_j   = relu(W1.T u_j + b1)                    # 128
    a3     = me*(y2@W3)+b3 ; a5 = sc*(y2@W5)+b5     # sigmoid-gated pair
    a4     = me*(y2@W4)+b4 ; a6 = sc*(y2@W6)+b6     # linear pair
    p      = sig(a3)*sig(a5)
    y      = FAILC + p*((a4+a6) - FAILC)

All heavy tensors travel as fp16 (verified 4.5x inside the 2e-2 tolerance
via numpy simulation): single-pass matmuls, no hi/lo split.  The outer3
features kt[r = s*5+n] = fe_s * res_n (s = m*6+o) are built from three
HOST-replicated fp16 layouts (fr/estep/res views) with two on-chip vector
multiplies per tile -- no on-chip replication matmuls or SBUF->SBUF DMAs.
All DMA loads are packed so each partition's bytes are contiguous in DRAM.
"""

import sys

if "/opt/trn_rl_repo" not in sys.path:
    sys.path.insert(0, "/opt/trn_rl_repo")

from contextlib import ExitStack

import numpy as np

import concourse.bass as bass
import concourse.mybir as mybir
import concourse.tile as tile
from concourse.bass_utils import run_bass_kernel_spmd

# Problem constants (hardcoded per the harness contract).
NCORES = 8
C, TASKS = 64, 64
T = C * TASKS                 # 4096
TC = T // NCORES              # 512 tasks per core
D_BB = 768
N_OUT = 150                   # 5*5*6 outer-product features
D_H = 128
E_N = 64                      # edge nodes
C_C, C_N = 4, 32              # cloud clusters / nodes
N_AGG = E_N + C_C + C_N       # 100
FAILC = -100.0
NTILE = TC // 128             # 4 task tiles of 128 per core

F32 = mybir.dt.float32
F16 = mybir.dt.float16


def _build_module():
    nc = bass.Bass()

    # Packed DRAM layouts (every partition's bytes contiguous):
    #   big1  [128, 3072]: bb k-blocks, col j*512+t = bb[task t, j*128+p]
    #         loaded as 3 chunks of 2 j-blocks so MMs start before it all lands
    #   wload [128,  905]: w1b blocks (0:768) | w1a'0 pad (768:896) | wh (896:900)
    #                      | bh broadcast (900:904) | b1 (904:905)
    #   p120  [120, 1536]: frrep0 | estrep0 | resrep0 (kt rows 0:120)
    #   p30   [30,  1664]: frrep1 | estrep1 | resrep1 | w1a'1 (kt rows 120:150)
    #   aggwa [100,  515]: aggT (0:512) | wa (512:515)
    big1 = nc.declare_dram_parameter("big1", [128, 6 * TC], F16, isOutput=False)
    wload = nc.declare_dram_parameter("wload", [128, 905], F16, isOutput=False)
    p120 = nc.declare_dram_parameter("p120", [120, 3 * TC], F16, isOutput=False)
    p30 = nc.declare_dram_parameter("p30", [30, 3 * TC + 128], F16, isOutput=False)
    aggwa = nc.declare_dram_parameter("aggwa", [N_AGG, TC + 3], F16, isOutput=False)
    out = nc.declare_dram_parameter("out", [128, NTILE], F32, isOutput=True)

    with tile.TileContext(nc) as tc, ExitStack() as ctx:
        sb = ctx.enter_context(tc.tile_pool(name="sb", bufs=1))
        psum = ctx.enter_context(tc.tile_pool(name="psum", bufs=1, space="PSUM"))

        # Preload the sigmoid ACT table early (overlaps the big DMAs) so the
        # real sigmoid near the kernel tail doesn't pay the 1.3us table load.
        sgw = sb.tile([32, 1], F32, tag="sgw")
        nc.vector.memset(sgw, 0.0)
        nc.scalar.activation(sgw, sgw, mybir.ActivationFunctionType.Sigmoid)

        # ---- input loads.  The sync queue (Q_I) gets DMA priority, so the
        # latency-critical tensors go there in consumption order; the kt
        # feature packs ride the scalar queue (Q_X) concurrently.
        aggwa_s = sb.tile([N_AGG, TC + 3], F16, tag="aggwa")
        nc.sync.dma_start(out=aggwa_s, in_=aggwa[:, :])
        wload_s = sb.tile([128, 905], F16, tag="wload")
        nc.sync.dma_start(out=wload_s, in_=wload[:, :])
        big1_s = sb.tile([128, 6 * TC], F16, tag="big1")
        nc.sync.dma_start(out=big1_s, in_=big1[:, :])
        p30_s = sb.tile([30, 3 * TC + 128], F16, tag="p30")
        nc.scalar.dma_start(out=p30_s, in_=p30[:, :])
        p120_s = sb.tile([120, 3 * TC], F16, tag="p120")
        nc.scalar.dma_start(out=p120_s, in_=p120[:, :])

        # ---- heads PSUM, task-major: [:, i, 0:4]=d3,d5,d4,d6  4:7=me,c1,c2
        psumS = psum.tile([128, NTILE, 7], F32, tag="psumS")
        # agg heads first: only need aggwa (arrives early, tensor idle).
        for i in range(NTILE):
            nc.tensor.matmul(
                psumS[:, i, 4:7],
                lhsT=aggwa_s[:, 128 * i : 128 * (i + 1)],
                rhs=aggwa_s[:, TC : TC + 3],
                start=True,
                stop=True,
            )

        # ---- main matmul: psumY[h,t] = sum_k W1[k,h] u[k,t] --------------
        psumY = psum.tile([128, TC], F32, tag="psumY")
        for j in range(6):
            nc.tensor.matmul(
                psumY,
                lhsT=wload_s[:, 128 * j : 128 * (j + 1)],
                rhs=big1_s[:, TC * j : TC * (j + 1)],
                start=(j == 0),
                stop=False,
            )

        # g4 = (me, sc, me, sc) per task -- built early, reads psumS[:,:,4:7]
        g4 = sb.tile([128, NTILE, 4], F32, tag="g4")
        nc.vector.tensor_copy(g4[:, :, 0:1], psumS[:, :, 4:5])
        nc.vector.tensor_copy(g4[:, :, 3:4], psumS[:, :, 6:7])
        nc.vector.tensor_mul(g4[:, :, 1:2], psumS[:, :, 5:6], g4[:, :, 3:4])
        nc.vector.tensor_copy(g4[:, :, 2:3], g4[:, :, 0:1])
        nc.vector.tensor_copy(g4[:, :, 3:4], g4[:, :, 1:2])

        # ---- outer3 features: kt = (fr_rep * estep_rep) * res_rep --------
        # kt1 first: p30 rides the earlier-draining part of the scalar queue.
        fe1 = sb.tile([30, TC], F16, tag="fe1")
        nc.vector.tensor_mul(fe1, p30_s[:, 0:TC], p30_s[:, TC : 2 * TC])
        kt1 = sb.tile([30, TC], F16, tag="kt1")
        nc.vector.tensor_mul(kt1, fe1, p30_s[:, 2 * TC : 3 * TC])
        fe0 = sb.tile([120, TC], F16, tag="fe0")
        nc.vector.tensor_mul(fe0, p120_s[:, 0:TC], p120_s[:, TC : 2 * TC])
        kt0 = sb.tile([120, TC], F16, tag="kt0")
        nc.vector.tensor_mul(kt0, fe0, p120_s[:, 2 * TC : 3 * TC])

        nc.tensor.matmul(
            psumY, lhsT=p30_s[:, 3 * TC : 3 * TC + 128], rhs=kt1,
            start=False, stop=False,
        )
        nc.tensor.matmul(
            psumY, lhsT=wload_s[0:120, 768:896], rhs=kt0, start=False, stop=True
        )

        # ---- relu (+b1) then y2 heads ------------------------------------
        y2T = sb.tile([128, TC], F16, tag="y2T")
        nc.scalar.activation(
            y2T, psumY, mybir.ActivationFunctionType.Relu,
            bias=wload_s[:, 904:905], scale=1.0,
        )
        for i in range(NTILE):
            nc.tensor.matmul(
                psumS[:, i, 0:4],
                lhsT=y2T[:, 128 * i : 128 * (i + 1)],
                rhs=wload_s[:, 896:900],
                start=True,
                stop=True,
            )

        # ---- combine ------------------------------------------------------
        av = sb.tile([128, NTILE, 4], F32, tag="av")
        nc.vector.tensor_mul(av, psumS[:, :, 0:4], g4)
        nc.vector.tensor_add(
            av, av,
            wload_s[:, 900:904].unsqueeze(1).broadcast_to([128, NTILE, 4]),
        )
        sg = sb.tile([128, NTILE, 2], F32, tag="sg")
        nc.scalar.activation(sg, av[:, :, 0:2], mybir.ActivationFunctionType.Sigmoid)

        # z = (a4 - FAILC) + a6  runs before sigmoid lands; then p*z + FAILC
        z = sb.tile([128, NTILE, 1], F32, tag="z")
        nc.vector.scalar_tensor_tensor(
            out=z,
            in0=av[:, :, 2:3],
            scalar=FAILC,
            in1=av[:, :, 3:4],
            op0=mybir.AluOpType.subtract,
            op1=mybir.AluOpType.add,
        )
        pv = sb.tile([128, NTILE, 1], F32, tag="pv")
        nc.vector.tensor_mul(pv, sg[:, :, 0:1], sg[:, :, 1:2])
        tt = sb.tile([128, NTILE, 1], F32, tag="tt")
        nc.vector.tensor_mul(tt, z, pv)
        outv = sb.tile([128, NTILE, 1], F32, tag="outv")
        nc.vector.tensor_scalar_add(outv, tt, FAILC)

        nc.gpsimd.dma_start(out=out[:, :], in_=outv[:, :, 0])

    return _split_sync_waits(nc)


def _split_sync_waits(nc, max_waits=1):
    """This container's walrus rejects >1 sem-wait per instruction
    ("Too many sync wait commands"); hoist extras onto same-engine NOPs."""
    nid = 0
    for f in nc.m.functions:
        for bb in f.blocks:
            new = []
            for inst in bb.instructions:
                si = inst.sync_info
                if si is None:
                    new.append(inst)
                    continue
                waits = list(si.on_wait or [])
                if len(waits) > max_waits:
                    for w in waits[:-max_waits]:
                        nop = mybir.InstNoOp(name=f"WSPL-{nid}", ins=[], outs=[])
                        nid += 1
                        nop.engine = inst.engine
                        nop.sync_info = mybir.SyncInfo(on_wait=[w], on_update=[])
                        new.append(nop)
                    inst.sync_info = mybir.SyncInfo(
                        on_wait=waits[-max_waits:], on_update=list(si.on_update or [])
                    )
                new.append(inst)
            bb.instructions = new
    return nc


_CACHED_NC = None


def _get_nc():
    global _CACHED_NC
    if _CACHED_NC is None:
        _CACHED_NC = _build_module()
    return _CACHED_NC


def _make_in_maps(inputs: dict) -> list[dict[str, np.ndarray]]:
    f32, f16 = np.float32, np.float16

    bb = np.asarray(inputs["backbone_y"], f32).reshape(T, D_BB)
    res = np.asarray(inputs["y_res"], f32).reshape(T, 5)
    fr = np.asarray(inputs["y_fr"], f32).reshape(T, 5)
    estep = np.asarray(inputs["y_estep"], f32).reshape(T, 6)
    enode = np.asarray(inputs["y_enode"], f32).reshape(T, E_N)
    ccl = np.asarray(inputs["y_ccluster"], f32).reshape(T, C_C)
    cnd = np.asarray(inputs["y_cnode"], f32).reshape(T, C_N)

    w1 = np.asarray(inputs["W1"], f32)
    # kt row r = s*5 + n  (s = m*6+o)  <-> original W1 row n*30 + s
    perm = np.array([n * 30 + s for s in range(30) for n in range(5)])
    w1ap = w1[perm]                       # [150, 128] permuted outer3 rows
    w1b = w1[N_OUT:]                      # [768, 128]
    w1bT = np.ascontiguousarray(
        w1b.reshape(6, 128, D_H).transpose(1, 0, 2).reshape(128, 6 * D_H)
    )
    w1a0 = np.zeros((128, D_H), f32)
    w1a0[0:120] = w1ap[0:120]
    w3 = np.asarray(inputs["W3"], f32).reshape(D_H, 1)
    w4 = np.asarray(inputs["W4"], f32).reshape(D_H, 1)
    w5 = np.asarray(inputs["W5"], f32).reshape(D_H, 1)
    w6 = np.asarray(inputs["W6"], f32).reshape(D_H, 1)
    # head col order: d3, d5 (sigmoid-gated), d4, d6 (linear)
    wh = np.concatenate([w3, w5, w4, w6], axis=1)           # [128, 4]
    bh = np.array(
        [
            float(np.asarray(inputs["b3"]).reshape(-1)[0]),
            float(np.asarray(inputs["b5"]).reshape(-1)[0]),
            float(np.asarray(inputs["b4"]).reshape(-1)[0]),
            float(np.asarray(inputs["b6"]).reshape(-1)[0]),
        ],
        f32,
    )
    b1 = np.asarray(inputs["b1"], f32).reshape(1, D_H)
    wload_h = np.ascontiguousarray(
        np.concatenate(
            [w1bT, w1a0, wh, np.broadcast_to(bh[None, :], (128, 4)), b1.T],
            axis=1,
        ).astype(f16)
    )

    wa = np.zeros((N_AGG, 3), f32)
    wa[0:E_N, 0] = 1.0 / E_N
    wa[E_N : E_N + C_C, 1] = 1.0
    wa[E_N + C_C :, 2] = 1.0

    # replication index maps for kt rows r = s*5+n
    r0 = np.arange(120)
    s0, n0 = r0 // 5, r0 % 5
    m0, o0 = s0 // 6, s0 % 6
    r1 = np.arange(30)
    s1, n1 = 24 + r1 // 5, r1 % 5
    m1, o1 = s1 // 6, s1 % 6

    in_maps = []
    for c in range(NCORES):
        sl = slice(c * TC, (c + 1) * TC)
        bbT_c = bb[sl].T                  # [768, TC]
        big1_c = np.ascontiguousarray(
            bbT_c.reshape(6, 128, TC).transpose(1, 0, 2).reshape(128, 6 * TC)
        ).astype(f16)
        fr_c, es_c, rs_c = fr[sl], estep[sl], res[sl]
        p120_c = np.ascontiguousarray(
            np.concatenate([fr_c[:, m0].T, es_c[:, o0].T, rs_c[:, n0].T], axis=1)
        ).astype(f16)
        p30_c = np.ascontiguousarray(
            np.concatenate(
                [fr_c[:, m1].T, es_c[:, o1].T, rs_c[:, n1].T, w1ap[120:150]],
                axis=1,
            )
        ).astype(f16)
        agg_c = np.concatenate([enode[sl], ccl[sl], cnd[sl]], axis=1).T  # [100,TC]
        aggwa_c = np.ascontiguousarray(
            np.concatenate([agg_c, wa], axis=1)
        ).astype(f16)
        in_maps.append(
            {
                "big1": big1_c,
                "wload": wload_h,
                "p120": p120_c,
                "p30": p30_c,
                "aggwa": aggwa_c,
            }
        )
    return in_maps


def _assemble(results: list[dict[str, np.ndarray]]) -> np.ndarray:
    parts = [np.asarray(results[c]["out"]).T.reshape(-1) for c in range(NCORES)]
    return np.concatenate(parts)[None, :].astype(np.float32)


def _run(inputs: dict, trace: bool = False):
    nc = _get_nc()
    in_maps = _make_in_maps(inputs)
    kres = run_bass_kernel_spmd(
        nc, in_maps, core_ids=list(range(NCORES)), trace=trace
    )
    return _assemble(kres.results), kres


def kernel(**inputs) -> np.ndarray:
    out, _ = _run(inputs)
    return out


# revision 16
# speedup vs baseline: 1.0359x; 1.0359x over previous
"""Trainium2 Bass kernel for nn_CriticHead (critic head over C*t tasks).

Contract: kernel(**inputs) takes the FULL unsharded inputs (as produced by
setup_inputs()) and returns the FULL [1, T] float32 output.  Internally the
work is sharded data-parallel over the leading cluster axis across 8
NeuronCores; the tiny MLP weights are replicated.

Math (per task j, verified against the reference):
    me_j   = mean(enode[j,:])                       # since y41 = y2 * me
    sc_j   = sum(ccl[j,:]) * sum(cnd[j,:])          # since y42 = y2 * sc
    u_j    = [outer3(res_j, fr_j, estep_j) (150) ; bb_j (768)]   # 918
    y2_j   = relu(W1.T u_j + b1)                    # 128
    a3     = me*(y2@W3)+b3 ; a5 = sc*(y2@W5)+b5     # sigmoid-gated pair
    a4     = me*(y2@W4)+b4 ; a6 = sc*(y2@W6)+b6     # linear pair
    p      = sig(a3)*sig(a5)
    y      = FAILC + p*((a4+a6) - FAILC)

All heavy tensors travel as fp16 (verified 4.5x inside the 2e-2 tolerance
via numpy simulation): single-pass matmuls, no hi/lo split.  The outer3
features kt[r = s*5+n] = fe_s * res_n (s = m*6+o) are built from three
HOST-replicated fp16 layouts (fr/estep/res views) with two on-chip vector
multiplies per tile -- no on-chip replication matmuls or SBUF->SBUF DMAs.
All DMA loads are packed so each partition's bytes are contiguous in DRAM.
"""

import sys

if "/opt/trn_rl_repo" not in sys.path:
    sys.path.insert(0, "/opt/trn_rl_repo")

from contextlib import ExitStack

import numpy as np

import concourse.bass as bass
import concourse.mybir as mybir
import concourse.tile as tile
from concourse.bass_utils import run_bass_kernel_spmd

# Problem constants (hardcoded per the harness contract).
NCORES = 8
C, TASKS = 64, 64
T = C * TASKS                 # 4096
TC = T // NCORES              # 512 tasks per core
D_BB = 768
N_OUT = 150                   # 5*5*6 outer-product features
D_H = 128
E_N = 64                      # edge nodes
C_C, C_N = 4, 32              # cloud clusters / nodes
N_AGG = E_N + C_C + C_N       # 100
FAILC = -100.0
NTILE = TC // 128             # 4 task tiles of 128 per core

F32 = mybir.dt.float32
F16 = mybir.dt.float16


def _build_module():
    nc = bass.Bass()

    # Packed DRAM layouts (every partition's bytes contiguous):
    #   big1  [128, 3072]: bb k-blocks, col j*512+t = bb[task t, j*128+p]
    #   wload [128,  905]: w1b blocks (0:768) | w1a'0 pad (768:896) | wh (896:900)
    #                      | bh broadcast (900:904) | b1 (904:905)
    #   p30c  [30,  1814]: fr30 | es30 | resrep1 | R0 (fe rep 30->120) |
    #                      R1 (fe rep 24:30 -> 30) | w1a'1
    #   resA  [120,  512]: resrep for kt rows 0:120
    #   aggwa [100,  515]: aggT (0:512) | wa (512:515)
    big1 = nc.declare_dram_parameter("big1", [128, 6 * TC], F16, isOutput=False)
    wload = nc.declare_dram_parameter("wload", [128, 905], F16, isOutput=False)
    p30c = nc.declare_dram_parameter("p30c", [30, 1814], F16, isOutput=False)
    resA = nc.declare_dram_parameter("resA", [120, TC], F16, isOutput=False)
    aggwa = nc.declare_dram_parameter("aggwa", [N_AGG, TC + 3], F16, isOutput=False)
    out = nc.declare_dram_parameter("out", [128, NTILE], F32, isOutput=True)

    with tile.TileContext(nc) as tc, ExitStack() as ctx:
        sb = ctx.enter_context(tc.tile_pool(name="sb", bufs=1))
        psum = ctx.enter_context(tc.tile_pool(name="psum", bufs=1, space="PSUM"))

        # Preload the sigmoid ACT table early (overlaps the big DMAs) so the
        # real sigmoid near the kernel tail doesn't pay the 1.3us table load.
        sgw = sb.tile([32, 1], F32, tag="sgw")
        nc.vector.memset(sgw, 0.0)
        nc.scalar.activation(sgw, sgw, mybir.ActivationFunctionType.Sigmoid)

        # ---- input loads.  The sync queue (Q_I) gets the fastest service,
        # so it carries the critical chain in consumption order; the agg
        # and resA packs ride the scalar queue (Q_X) concurrently.
        p30c_s = sb.tile([30, 1814], F16, tag="p30c")
        nc.sync.dma_start(out=p30c_s, in_=p30c[:, :])
        wload_s = sb.tile([128, 905], F16, tag="wload")
        nc.sync.dma_start(out=wload_s, in_=wload[:, :])
        big1_s = sb.tile([128, 6 * TC], F16, tag="big1")
        nc.sync.dma_start(out=big1_s, in_=big1[:, :])
        aggwa_s = sb.tile([N_AGG, TC + 3], F16, tag="aggwa")
        nc.scalar.dma_start(out=aggwa_s, in_=aggwa[:, :])
        resA_s = sb.tile([120, TC], F16, tag="resA")
        nc.scalar.dma_start(out=resA_s, in_=resA[:, :])

        # ---- outer3 features: fe = fr*estep [30], PE-replicate to kt rows,
        # then kt = fe_rep * res_rep.
        fe30 = sb.tile([30, TC], F16, tag="fe30")
        nc.vector.tensor_mul(fe30, p30c_s[:, 0:TC], p30c_s[:, TC : 2 * TC])
        ps_f0 = psum.tile([120, TC], F32, tag="ps_f0")
        nc.tensor.matmul(
            ps_f0, lhsT=p30c_s[:, 1536:1656], rhs=fe30, start=True, stop=True
        )
        ps_f1 = psum.tile([30, TC], F32, tag="ps_f1")
        nc.tensor.matmul(
            ps_f1, lhsT=p30c_s[:, 1656:1686], rhs=fe30, start=True, stop=True
        )
        kt1 = sb.tile([30, TC], F16, tag="kt1")
        nc.vector.tensor_mul(kt1, ps_f1, p30c_s[:, 2 * TC : 3 * TC])
        kt0 = sb.tile([120, TC], F16, tag="kt0")
        nc.vector.tensor_mul(kt0, ps_f0, resA_s)

        # ---- heads PSUM, task-major: [:, i, 0:4]=d3,d5,d4,d6  4:7=me,c1,c2
        psumS = psum.tile([128, NTILE, 7], F32, tag="psumS")
        psumY = psum.tile([128, TC], F32, tag="psumY")

        # kt1 into the accumulator as soon as it exists (tensor idle window)
        nc.tensor.matmul(
            psumY, lhsT=p30c_s[:, 1686:1814], rhs=kt1, start=True, stop=False
        )
        # agg heads: need only aggwa (early on the scalar queue)
        for i in range(NTILE):
            nc.tensor.matmul(
                psumS[:, i, 4:7],
                lhsT=aggwa_s[:, 128 * i : 128 * (i + 1)],
                rhs=aggwa_s[:, TC : TC + 3],
                start=True,
                stop=True,
            )
        # main contraction over bb, then kt0 last (stop)
        for j in range(6):
            nc.tensor.matmul(
                psumY,
                lhsT=wload_s[:, 128 * j : 128 * (j + 1)],
                rhs=big1_s[:, TC * j : TC * (j + 1)],
                start=False,
                stop=False,
            )
        nc.tensor.matmul(
            psumY, lhsT=wload_s[0:120, 768:896], rhs=kt0, start=False, stop=True
        )

        # g4 = (me, sc, me, sc) per task -- built early, reads psumS[:,:,4:7]
        g4 = sb.tile([128, NTILE, 4], F32, tag="g4")
        nc.vector.tensor_copy(g4[:, :, 0:1], psumS[:, :, 4:5])
        nc.vector.tensor_copy(g4[:, :, 3:4], psumS[:, :, 6:7])
        nc.vector.tensor_mul(g4[:, :, 1:2], psumS[:, :, 5:6], g4[:, :, 3:4])
        nc.vector.tensor_copy(g4[:, :, 2:3], g4[:, :, 0:1])
        nc.vector.tensor_copy(g4[:, :, 3:4], g4[:, :, 1:2])

        # ---- relu (+b1) then y2 heads ------------------------------------
        y2T = sb.tile([128, TC], F16, tag="y2T")
        nc.scalar.activation(
            y2T, psumY, mybir.ActivationFunctionType.Relu,
            bias=wload_s[:, 904:905], scale=1.0,
        )
        for i in range(NTILE):
            nc.tensor.matmul(
                psumS[:, i, 0:4],
                lhsT=y2T[:, 128 * i : 128 * (i + 1)],
                rhs=wload_s[:, 896:900],
                start=True,
                stop=True,
            )

        # ---- combine ------------------------------------------------------
        av = sb.tile([128, NTILE, 4], F32, tag="av")
        nc.vector.tensor_mul(av, psumS[:, :, 0:4], g4)
        nc.vector.tensor_add(
            av, av,
            wload_s[:, 900:904].unsqueeze(1).broadcast_to([128, NTILE, 4]),
        )
        sg = sb.tile([128, NTILE, 2], F32, tag="sg")
        nc.scalar.activation(sg, av[:, :, 0:2], mybir.ActivationFunctionType.Sigmoid)

        # z = (a4 - FAILC) + a6  runs before sigmoid lands; then p*z + FAILC
        z = sb.tile([128, NTILE, 1], F32, tag="z")
        nc.vector.scalar_tensor_tensor(
            out=z,
            in0=av[:, :, 2:3],
            scalar=FAILC,
            in1=av[:, :, 3:4],
            op0=mybir.AluOpType.subtract,
            op1=mybir.AluOpType.add,
        )
        pv = sb.tile([128, NTILE, 1], F32, tag="pv")
        nc.vector.tensor_mul(pv, sg[:, :, 0:1], sg[:, :, 1:2])
        tt = sb.tile([128, NTILE, 1], F32, tag="tt")
        nc.vector.tensor_mul(tt, z, pv)
        outv = sb.tile([128, NTILE, 1], F32, tag="outv")
        nc.vector.tensor_scalar_add(outv, tt, FAILC)

        nc.gpsimd.dma_start(out=out[:, :], in_=outv[:, :, 0])

    return _split_sync_waits(nc)


def _split_sync_waits(nc, max_waits=1):
    """This container's walrus rejects >1 sem-wait per instruction
    ("Too many sync wait commands"); hoist extras onto same-engine NOPs."""
    nid = 0
    for f in nc.m.functions:
        for bb in f.blocks:
            new = []
            for inst in bb.instructions:
                si = inst.sync_info
                if si is None:
                    new.append(inst)
                    continue
                waits = list(si.on_wait or [])
                if len(waits) > max_waits:
                    for w in waits[:-max_waits]:
                        nop = mybir.InstNoOp(name=f"WSPL-{nid}", ins=[], outs=[])
                        nid += 1
                        nop.engine = inst.engine
                        nop.sync_info = mybir.SyncInfo(on_wait=[w], on_update=[])
                        new.append(nop)
                    inst.sync_info = mybir.SyncInfo(
                        on_wait=waits[-max_waits:], on_update=list(si.on_update or [])
                    )
                new.append(inst)
            bb.instructions = new
    return nc


_CACHED_NC = None


def _get_nc():
    global _CACHED_NC
    if _CACHED_NC is None:
        _CACHED_NC = _build_module()
    return _CACHED_NC


def _make_in_maps(inputs: dict) -> list[dict[str, np.ndarray]]:
    f32, f16 = np.float32, np.float16

    bb = np.asarray(inputs["backbone_y"], f32).reshape(T, D_BB)
    res = np.asarray(inputs["y_res"], f32).reshape(T, 5)
    fr = np.asarray(inputs["y_fr"], f32).reshape(T, 5)
    estep = np.asarray(inputs["y_estep"], f32).reshape(T, 6)
    enode = np.asarray(inputs["y_enode"], f32).reshape(T, E_N)
    ccl = np.asarray(inputs["y_ccluster"], f32).reshape(T, C_C)
    cnd = np.asarray(inputs["y_cnode"], f32).reshape(T, C_N)

    w1 = np.asarray(inputs["W1"], f32)
    # kt row r = s*5 + n  (s = m*6+o)  <-> original W1 row n*30 + s
    perm = np.array([n * 30 + s for s in range(30) for n in range(5)])
    w1ap = w1[perm]                       # [150, 128] permuted outer3 rows
    w1b = w1[N_OUT:]                      # [768, 128]
    w1bT = np.ascontiguousarray(
        w1b.reshape(6, 128, D_H).transpose(1, 0, 2).reshape(128, 6 * D_H)
    )
    w1a0 = np.zeros((128, D_H), f32)
    w1a0[0:120] = w1ap[0:120]
    w3 = np.asarray(inputs["W3"], f32).reshape(D_H, 1)
    w4 = np.asarray(inputs["W4"], f32).reshape(D_H, 1)
    w5 = np.asarray(inputs["W5"], f32).reshape(D_H, 1)
    w6 = np.asarray(inputs["W6"], f32).reshape(D_H, 1)
    # head col order: d3, d5 (sigmoid-gated), d4, d6 (linear)
    wh = np.concatenate([w3, w5, w4, w6], axis=1)           # [128, 4]
    bh = np.array(
        [
            float(np.asarray(inputs["b3"]).reshape(-1)[0]),
            float(np.asarray(inputs["b5"]).reshape(-1)[0]),
            float(np.asarray(inputs["b4"]).reshape(-1)[0]),
            float(np.asarray(inputs["b6"]).reshape(-1)[0]),
        ],
        f32,
    )
    b1 = np.asarray(inputs["b1"], f32).reshape(1, D_H)
    wload_h = np.ascontiguousarray(
        np.concatenate(
            [w1bT, w1a0, wh, np.broadcast_to(bh[None, :], (128, 4)), b1.T],
            axis=1,
        ).astype(f16)
    )

    wa = np.zeros((N_AGG, 3), f32)
    wa[0:E_N, 0] = 1.0 / E_N
    wa[E_N : E_N + C_C, 1] = 1.0
    wa[E_N + C_C :, 2] = 1.0

    # fe row s = m*6+o holds fr_m*estep_o; kt row r = s*5+n
    s_all = np.arange(30)
    m_all, o_all = s_all // 6, s_all % 6
    # R0 [30,120]: fe row s -> kt0 rows r (s == r//5); R1 [30,30] for s>=24
    r0 = np.arange(120)
    R0 = (s_all[:, None] == r0[None, :] // 5).astype(f32)
    r1 = np.arange(30)
    R1 = (s_all[:, None] == 24 + r1[None, :] // 5).astype(f32)

    in_maps = []
    for c in range(NCORES):
        sl = slice(c * TC, (c + 1) * TC)
        bbT_c = bb[sl].T                  # [768, TC]
        big1_c = np.ascontiguousarray(
            bbT_c.reshape(6, 128, TC).transpose(1, 0, 2).reshape(128, 6 * TC)
        ).astype(f16)
        fr_c, es_c, rs_c = fr[sl], estep[sl], res[sl]
        p30c_c = np.ascontiguousarray(
            np.concatenate(
                [
                    fr_c[:, m_all].T,           # fr30
                    es_c[:, o_all].T,           # es30
                    rs_c[:, r1 % 5].T,          # resrep1 (kt rows 120:150)
                    R0,
                    R1,
                    w1ap[120:150],
                ],
                axis=1,
            )
        ).astype(f16)
        resA_c = np.ascontiguousarray(rs_c[:, r0 % 5].T).astype(f16)
        agg_c = np.concatenate([enode[sl], ccl[sl], cnd[sl]], axis=1).T  # [100,TC]
        aggwa_c = np.ascontiguousarray(
            np.concatenate([agg_c, wa], axis=1)
        ).astype(f16)
        in_maps.append(
            {
                "big1": big1_c,
                "wload": wload_h,
                "p30c": p30c_c,
                "resA": resA_c,
                "aggwa": aggwa_c,
            }
        )
    return in_maps


def _assemble(results: list[dict[str, np.ndarray]]) -> np.ndarray:
    parts = [np.asarray(results[c]["out"]).T.reshape(-1) for c in range(NCORES)]
    return np.concatenate(parts)[None, :].astype(np.float32)


def _run(inputs: dict, trace: bool = False):
    nc = _get_nc()
    in_maps = _make_in_maps(inputs)
    kres = run_bass_kernel_spmd(
        nc, in_maps, core_ids=list(range(NCORES)), trace=trace
    )
    return _assemble(kres.results), kres


def kernel(**inputs) -> np.ndarray:
    out, _ = _run(inputs)
    return out
